# revision 46
# baseline (speedup 1.0000x reference)
"""nn_Encoder TRN2 kernel — data-parallel over batch on 8 NeuronCores.

Per core (16 samples, T=4096 tokens):
  conv  : im2col patches [147, T] (host-prepped) x w0 -> prelu -> H
  L1..L3: 1x1 conv (f32r matmul) -> BN (global stats via AllReduce) -> prelu,
          activations kept in SBUF, pre-BN y overwrites H in place
  mixer : +pos, prelu, x wm.T -> [token, 512] tiles
  perm  : per-sample one-hot permutation matmul + bias -> output

All matmuls run as float32r (full PE rate, ~1.5e-4 rel err measured).
"""
from contextlib import ExitStack

import numpy as np
import concourse.bass as bass
from concourse import bacc
import concourse.tile as tile
import concourse.mybir as mybir
from concourse.bass_utils import run_bass_kernel_spmd
from concourse.tile_rust import add_dep_helper

F32 = mybir.dt.float32
F32R = mybir.dt.float32r
AFT = mybir.ActivationFunctionType
ADD = mybir.AluOpType.add

N_CORES = 8
B, CIN, IMG, KK = 128, 3, 112, 7
C, HID, HW_ = 1024, 512, 256
EPS = 1e-5
BL = B // N_CORES          # 16 samples per core
T = BL * HW_               # 4096 tokens per core
KP = CIN * KK * KK         # 147 patch elems
NDT = C // 128             # 8 channel tiles
NTB = T // 512             # 8 token blocks of 512
TS = bass.ts

_cached = {}


def _build(n_cores=N_CORES, dbg=False):
    nc = bacc.Bacc("TRN2", num_devices=n_cores)
    dbg_d = {}
    if dbg:
        dbg_d["ss"] = nc.dram_tensor("dbg_ss", [128, 3, 2, NDT], F32,
                                     kind="ExternalOutput")
        for st in ("conv", "y0", "l0", "l1", "l2", "enc"):
            dbg_d[st] = nc.dram_tensor(f"dbg_{st}", [C, T], F32R,
                                       kind="ExternalOutput")

    last_dump = {}

    def dump(st, h, nc):
        if not dbg:
            return
        for ct in range(NDT):
            for tb in range(NTB):
                ins = nc.sync.dma_start(
                    dbg_d[st].ap()[ct * 128:(ct + 1) * 128, TS(tb, 512)],
                    h[ct][tb][:])
                last_dump[(ct, tb)] = ins

    xp_d = nc.dram_tensor("xp", [KP, T], F32R, kind="ExternalInput")
    w0p_d = nc.dram_tensor("w0p", [KP, C], F32R, kind="ExternalInput")
    wt_d = [nc.dram_tensor(f"wt{l}", [C, C], F32R, kind="ExternalInput")
            for l in (1, 2, 3)]
    wmt_d = nc.dram_tensor("wmt", [C, HID], F32R, kind="ExternalInput")
    ph_d = nc.dram_tensor("ph", [BL, 2, 2, 128, 128], F32R, kind="ExternalInput")
    post_d = nc.dram_tensor("post", [128, NDT, HW_], F32R, kind="ExternalInput")
    bmb_d = nc.dram_tensor("bmb", [128, HID], F32, kind="ExternalInput")
    b0c_d = nc.dram_tensor("b0c", [128, NDT], F32, kind="ExternalInput")
    gc_d = [nc.dram_tensor(f"g{l}c", [128, NDT], F32, kind="ExternalInput")
            for l in (1, 2, 3)]
    btc_d = [nc.dram_tensor(f"bt{l}c", [128, NDT], F32, kind="ExternalInput")
             for l in (1, 2, 3)]
    al0_d = nc.dram_tensor("al0", [128, 1], F32, kind="ExternalInput")
    alp_d = [nc.dram_tensor(f"al{l}", [128, 1], F32, kind="ExternalInput")
             for l in (1, 2, 3)]
    alm_d = nc.dram_tensor("alm", [128, 1], F32, kind="ExternalInput")
    out_d = nc.dram_tensor("out", [T, HID], F32, kind="ExternalOutput")

    with tile.TileContext(nc) as tc:
        with tc.tile_pool(name="main", bufs=1) as mp, \
             tc.tile_pool(name="psum", bufs=8, space="PSUM") as pp, \
             tc.tile_pool(name="dram", bufs=1, space="DRAM") as dp:

            # persistent activation tiles: h[ct][tb] = [128, 512]
            h = [[mp.tile([128, 512], F32R, name=f"h_{ct}_{tb}", tag=f"h_{ct}_{tb}")
                  for tb in range(NTB)] for ct in range(NDT)]

            _wp_stack = ExitStack()
            wp = _wp_stack.enter_context(tc.tile_pool(name="wp", bufs=1))
            if True:
                # conv phase: stream im2col blocks, weights resident.
                # DMA order matters: conv operands first (HWDGE), big weight
                # prefetch on SWDGE so it doesn't block the stream.
                with tc.tile_pool(name="xp", bufs=4) as xpool:
                    w_s = wp.tile([128, NDT, C], F32R, name="w_s", tag="w")
                    wsrc = wt_d[0].ap().rearrange("(ct p) d -> p ct d", p=128)
                    w0m = xpool.tile([128, C], F32R, name="w0m", bufs=1)
                    w0t = xpool.tile([KP - 128, C], F32R, name="w0t", bufs=1)
                    b0c_s = mp.tile([128, NDT], F32, name="b0c_s")
                    al0_s = mp.tile([128, 1], F32, name="al0_s")
                    for tb in range(NTB):
                        xm = xpool.tile([128, 512], F32R, name="xm")
                        xdma = nc.sync.dma_start(xm[:],
                                                 xp_d.ap()[0:128, TS(tb, 512)])
                        if tb == 0:
                            # main-matmul weights first: the very first MM
                            # needs only xm0 + w0m
                            nc.sync.dma_start(w0m[:], w0p_d.ap()[0:128, :])
                        xt = xpool.tile([KP - 128, 512], F32R, name="xt")
                        nc.sync.dma_start(xt[:], xp_d.ap()[128:KP, TS(tb, 512)])
                        if tb == 0:
                            nc.sync.dma_start(w0t[:], w0p_d.ap()[128:KP, :])
                            nc.sync.dma_start(b0c_s[:], b0c_d.ap())
                            nc.sync.dma_start(al0_s[:], al0_d.ap())
                        # prefetch L1 weights during conv (SWDGE), one c-tile
                        # per token block, paced behind the stream tile so the
                        # weight data never outruns conv operands in the pipe
                        wdma = nc.gpsimd.dma_start(w_s[:, tb, :], wsrc[:, tb, :])
                        add_dep_helper(wdma.ins, xdma.ins,
                                       reason="pace weight prefetch")
                        for dt in range(NDT):
                            ps = pp.tile([128, 512], F32, name="ps", tag="ps")
                            nc.tensor.matmul(ps[:], w0m[:, TS(dt, 128)], xm[:],
                                             start=True, stop=False)
                            nc.tensor.matmul(ps[:], w0t[:, TS(dt, 128)], xt[:],
                                             start=False, stop=True)
                            if dt < 2:
                                # conv is ACT-bound; route two drains per
                                # block through DVE: z = y+b, h = max(z, a*z)
                                zt = xpool.tile([128, 512], F32, name="zt",
                                                tag="zt", bufs=3)
                                nc.vector.tensor_scalar_add(
                                    zt[:], ps[:], b0c_s[:, dt:dt + 1])
                                nc.vector.scalar_tensor_tensor(
                                    h[dt][tb][:], zt[:], al0_s[:], zt[:],
                                    op0=mybir.AluOpType.mult,
                                    op1=mybir.AluOpType.max)
                            else:
                                nc.scalar.activation(
                                    h[dt][tb][:], ps[:], AFT.Prelu,
                                    bias=b0c_s[:, dt:dt + 1], scale=1.0,
                                    alpha=al0_s[:])

                    # per-layer consts, needed from the first BN boundary on
                    al_s = []
                    for l in range(3):
                        t_ = mp.tile([128, 1], F32, name=f"al{l + 1}_s")
                        nc.sync.dma_start(t_[:], alp_d[l].ap())
                        al_s.append(t_)
                    alm_s = mp.tile([128, 1], F32, name="alm_s")
                    nc.sync.dma_start(alm_s[:], alm_d.ap())
                    gc_s, btc_s = [], []
                    for l in range(3):
                        g_ = mp.tile([128, NDT], F32, name=f"g{l + 1}_s")
                        nc.sync.dma_start(g_[:], gc_d[l].ap())
                        gc_s.append(g_)
                        b_ = mp.tile([128, NDT], F32, name=f"bt{l + 1}_s")
                        nc.sync.dma_start(b_[:], btc_d[l].ap())
                        btc_s.append(b_)

                dump("conv", h, nc)
                _mixw_stack = ExitStack()

                # L1..L3
                recs = mp.tile([128, NDT, NTB, 6], F32, name="recs", tag="recs")
                for l in range(3):
                    if l == 1:
                        # mixer weights: load well before the mixer phase,
                        # on the ACT HWDGE ring (right-side pool)
                        mixw = _mixw_stack.enter_context(
                            tc.tile_pool(name="mixw", bufs=1, side="right"))
                        wmt_s = mixw.tile([128, NDT, HID], F32R, name="wmt_s")
                        wmsrc = wmt_d.ap().rearrange("(ct p) d -> p ct d",
                                                     p=128)
                        for ct in range(NDT):
                            nc.scalar.dma_start(wmt_s[:, ct, :], wmsrc[:, ct, :])
                    # pass 1: y = W h (pre-BN), overwrite h in place, collect stats
                    def _drains(tb, ps_list, last_mm, after=None,
                                pings=None):
                        # in-place overwrite: explicit WAR dep on the last MM
                        # of this token block (PE completes in order)
                        for dt in range(NDT):
                            src_t = (pings[dt] if pings and dt in pings
                                     else ps_list[dt])
                            cp = nc.vector.tensor_copy(h[dt][tb][:],
                                                       src_t[:])
                            add_dep_helper(cp.ins, last_mm.ins,
                                           reason="inplace h WAR")
                            if after is not None:
                                add_dep_helper(cp.ins, after.ins, sync=False,
                                               reason="drains after AR pack")
                            if dbg and (dt, tb) in last_dump:
                                add_dep_helper(cp.ins, last_dump[(dt, tb)].ins,
                                               reason="dbg dump WAR")

                    held = None
                    lmv = mp.tile([128, NDT, 2], F32, name="lmv", tag="lmv")
                    arp = mp.tile([128, NDT, 2], F32, name="arp", tag="arp")
                    m2 = mp.tile([128, NDT], F32, name="m2", tag="m2")
                    for tb in range(NTB):
                        ps_list = []
                        pings = {}
                        last_mm = None
                        for dt in range(NDT):
                            ps = pp.tile([128, 512], F32, name="ps", tag="ps")
                            for ct in range(NDT):
                                last_mm = nc.tensor.matmul(
                                    ps[:], w_s[:, ct, TS(dt, 128)],
                                    h[ct][tb][:],
                                    start=(ct == 0), stop=(ct == NDT - 1))
                            # dt=7's record would sit between the last MM and
                            # the first drain; defer it so a PSUM bank frees
                            # as early as possible for the next token block
                            if dt < NDT - 1 or tb == NTB - 1:
                                nc.vector.bn_stats(recs[:, dt, tb, :], ps[:])
                            if tb == NTB - 1:
                                # all 8 records for this dt now exist:
                                # aggregate AND pack the AllReduce payload
                                # slice now, overlapping the next dt's MMs
                                nc.vector.bn_aggr(lmv[:, dt, :],
                                                  recs[:, dt, :, :])
                                nc.vector.tensor_mul(m2[:, dt:dt + 1],
                                                     lmv[:, dt, 0:1],
                                                     lmv[:, dt, 0:1])
                                nc.vector.tensor_add(m2[:, dt:dt + 1],
                                                     lmv[:, dt, 1:2],
                                                     m2[:, dt:dt + 1])
                                nc.vector.tensor_scalar_mul(
                                    arp[:, dt, 0:1], lmv[:, dt, 0:1],
                                    1.0 / n_cores)
                                nc.vector.tensor_scalar_mul(
                                    arp[:, dt, 1:2], m2[:, dt:dt + 1],
                                    1.0 / n_cores)
                            # stage the first two groups out of PSUM right
                            # away: their banks free mid-block, so the next
                            # token block's first matmuls never wait
                            if dt < 2:
                                pg = mp.tile([128, 512], F32R, name="ping",
                                             tag="ping", bufs=4)
                                nc.vector.tensor_copy(pg[:], ps[:])
                                pings[dt] = pg
                            ps_list.append(ps)
                        if tb < NTB - 1:
                            _drains(tb, ps_list, last_mm, pings=pings)
                            nc.vector.bn_stats(recs[:, NDT - 1, tb, :],
                                               ps_list[NDT - 1][:])
                        else:
                            # last block: stats go to the AllReduce first;
                            # drains are emitted after the collective trigger
                            held = (tb, ps_list, last_mm, pings)
                    if l == 0:
                        dump("y0", h, nc)
                    # AllReduce (payload already packed per-dt above;
                    # emitted before the weight prefetch so the trigger isn't
                    # queued behind SWDGE descriptor generation on gpsimd)
                    ar_in = dp.tile([128, NDT * 2], F32, name=f"arin{l}")
                    ar_out = dp.tile([128, NDT * 2], F32, name=f"arout{l}")
                    bdma = nc.sync.dma_start(
                        ar_in[:], arp[:].rearrange("p a b -> p (a b)"))
                    cc = nc.gpsimd.collective_compute(
                        "AllReduce", ADD,
                        replica_groups=[list(range(n_cores))],
                        ins=[ar_in.opt()], outs=[ar_out.opt()])
                    # prefetch next layer's weights (slot frees at last MM);
                    # nosync edge keeps the trigger ahead of descgen on gpsimd
                    if l < 2:
                        w_s = wp.tile([128, NDT, C], F32R, name="w_s", tag="w")
                        wsrc = wt_d[l + 1].ap().rearrange("(ct p) d -> p ct d",
                                                          p=128)
                        for ct in range(NDT):
                            wdma = nc.gpsimd.dma_start(w_s[:, ct, :],
                                                       wsrc[:, ct, :])
                            add_dep_helper(wdma.ins, cc.ins, sync=False,
                                           reason="trigger before descgen")
                    _drains(held[0], held[1], held[2], after=bdma,
                            pings=held[3])
                    gst = mp.tile([128, NDT, 2], F32, name="gst", tag="gst")
                    nc.sync.dma_start(gst[:].rearrange("p a b -> p (a b)"), ar_out[:])
                    # finalize: scale = g*rsqrt(var+eps), shift = bt -
                    # mean*scale.  dt=0's [128,1] slice is computed first so
                    # pass-2 can start while the remaining dt finalize.
                    gmean = gst[:, :, 0]
                    gvar = mp.tile([128, NDT], F32, name="gvar", tag="gvar")
                    stdv = mp.tile([128, NDT], F32, name="stdv", tag="stdv")
                    inv = mp.tile([128, NDT], F32, name="inv", tag="inv")
                    scl = mp.tile([128, NDT], F32, name="scl", tag="scl")
                    shf = mp.tile([128, NDT], F32, name="shf", tag="shf")
                    for sl in (slice(0, 1), slice(1, NDT)):
                        nc.vector.tensor_mul(m2[:, sl], gmean[:, sl],
                                             gmean[:, sl])
                        nc.vector.tensor_sub(gvar[:, sl], gst[:, sl, 1],
                                             m2[:, sl])
                        nc.vector.tensor_scalar_add(gvar[:, sl], gvar[:, sl],
                                                    EPS)
                        nc.scalar.activation(stdv[:, sl], gvar[:, sl], AFT.Sqrt)
                        nc.vector.reciprocal(inv[:, sl], stdv[:, sl])
                        nc.vector.tensor_mul(scl[:, sl], gc_s[l][:, sl],
                                             inv[:, sl])
                        nc.vector.tensor_mul(m2[:, sl], gmean[:, sl],
                                             scl[:, sl])
                        nc.vector.tensor_sub(shf[:, sl], btc_s[l][:, sl],
                                             m2[:, sl])
                    if dbg:
                        nc.sync.dma_start(dbg_d["ss"].ap()[:, l, 0, :], scl[:])
                        nc.sync.dma_start(dbg_d["ss"].ap()[:, l, 1, :], shf[:])
                    # pass 2: h = prelu(y*scale + shift). For L3 it is
                    # deferred into the mixer phase, fused with pos/prelu-am.
                    if l == 2:
                        scl3, shf3 = scl, shf
                    else:
                        for tb in range(NTB):
                            for dt in range(NDT):
                                act = nc.scalar.activation(
                                    h[dt][tb][:], h[dt][tb][:], AFT.Prelu,
                                    bias=shf[:, dt:dt + 1],
                                    scale=scl[:, dt:dt + 1],
                                    alpha=al_s[l][:])
                                if dbg and (dt, tb) in last_dump:
                                    add_dep_helper(act.ins,
                                                   last_dump[(dt, tb)].ins,
                                                   reason="dbg dump WAR")
                        dump(f"l{l}", h, nc)

            _wp_stack.close()
            # mixer + permutation phase.  These loads become runnable the
            # moment the weight pool releases (= L3's last MM); pace them
            # behind the L3 AllReduce bounce-out so they don't delay it.
            with tc.tile_pool(name="mix", bufs=1, side="right") as mxp, \
                 tc.tile_pool(name="ph", bufs=3, side="right") as php_pool:
                post_s = mxp.tile([128, NDT, HW_], F32R, name="post_s")
                d1 = nc.scalar.dma_start(post_s[:], post_d.ap())
                add_dep_helper(d1.ins, bdma.ins, reason="after L3 AR bounce")
                bmb_s = mxp.tile([128, HID], F32, name="bmb_s")
                d2 = nc.scalar.dma_start(bmb_s[:], bmb_d.ap())
                add_dep_helper(d2.ins, bdma.ins, reason="after L3 AR bounce")
                # per token block: fused chains (L3 pass-2 -> +pos ->
                # prelu-am), then immediately the block's mixer + permutation
                # matmuls, so drains interleave with chain work in the DVE
                # FIFO instead of queueing behind all of it
                def chain(tb):
                    for ct in range(NDT):
                        act = nc.scalar.activation(
                            h[ct][tb][:], h[ct][tb][:], AFT.Prelu,
                            bias=shf3[:, ct:ct + 1], scale=scl3[:, ct:ct + 1],
                            alpha=al_s[2][:])
                        if dbg and (ct, tb) in last_dump:
                            add_dep_helper(act.ins, last_dump[(ct, tb)].ins,
                                           reason="dbg dump WAR")
                        hv = h[ct][tb][:].rearrange("p (s j) -> p s j", j=HW_)
                        pv = post_s[:, ct, :]
                        pb = bass.AP(pv.tensor, pv.offset,
                                     [list(pv.ap[0]), [0, 512 // HW_],
                                      list(pv.ap[-1])])
                        pos_eng = nc.vector if ct % 4 == 3 else nc.gpsimd
                        pos_eng.tensor_tensor(hv, hv, pb, op=ADD)
                        if ct % 2 == 0:
                            nc.scalar.activation(h[ct][tb][:], h[ct][tb][:],
                                                 AFT.Prelu, bias=0.0,
                                                 scale=1.0, alpha=alm_s[:])
                        else:
                            nc.vector.scalar_tensor_tensor(
                                h[ct][tb][:], h[ct][tb][:], alm_s[:],
                                h[ct][tb][:], op0=mybir.AluOpType.mult,
                                op1=mybir.AluOpType.max)

                for tb in range(NTB):
                    chain(tb)
                    for s in (tb * 2, tb * 2 + 1):   # two samples per block
                        mx = []
                        for half in range(2):
                            st = s * 2 + half
                            k = st % 4
                            ps = pp.tile([128, 512], F32, name="ps", tag="ps")
                            for ct in range(NDT):
                                nc.tensor.matmul(
                                    ps[:], h[ct][tb][:, TS(k, 128)],
                                    wmt_s[:, ct, :], start=(ct == 0),
                                    stop=(ct == NDT - 1))
                            m_ = mxp.tile([128, HID], F32R, name="mx", bufs=6)
                            nc.vector.tensor_copy(m_[:], ps[:])
                            mx.append(m_)
                        php = php_pool.tile([128, 2, 2, 128], F32R, name="php")
                        pdma = nc.scalar.dma_start(
                            php[:],
                            ph_d.ap()[s].rearrange("kt mt ti to -> ti kt mt to"))
                        if s < 2:
                            add_dep_helper(pdma.ins, bdma.ins,
                                           reason="after L3 AR bounce")
                        for mt in range(2):
                            pso = pp.tile([128, 512], F32, name="ps", tag="ps")
                            nc.tensor.matmul(pso[:], php[:, 0, mt, :],
                                             mx[0][:],
                                             start=True, stop=False)
                            nc.tensor.matmul(pso[:], php[:, 1, mt, :],
                                             mx[1][:],
                                             start=False, stop=True)
                            ot = mxp.tile([128, HID], F32, name="ot", bufs=4)
                            nc.vector.tensor_add(ot[:], pso[:], bmb_s[:])
                            nc.sync.dma_start(
                                out_d.ap()[s * HW_ + mt * 128:
                                           s * HW_ + (mt + 1) * 128, :], ot[:])
                dump("enc", h, nc)

            _mixw_stack.close()

    nc.compile()
    return nc


def _prep_inputs(x, w0, b0, a0, w1, g1, bt1, p1, w2, g2, bt2, p2,
                 w3, g3, bt3, p3, pos, am, wm, bm, perm):
    """Host-side marshalling: shard + relayout. Returns in_maps for 8 cores."""
    f32 = np.float32
    com = {
        "w0p": np.ascontiguousarray(w0.reshape(C, KP).T, dtype=f32),
        "wt1": np.ascontiguousarray(w1.T, dtype=f32),
        "wt2": np.ascontiguousarray(w2.T, dtype=f32),
        "wt3": np.ascontiguousarray(w3.T, dtype=f32),
        "wmt": np.ascontiguousarray(wm.T, dtype=f32),
        "post": np.ascontiguousarray(
            pos[0].T.reshape(NDT, 128, HW_).transpose(1, 0, 2), dtype=f32),
        "bmb": np.tile(bm.astype(f32), (128, 1)),
        "b0c": np.ascontiguousarray(b0.reshape(NDT, 128).T, dtype=f32),
        "al0": np.tile(np.asarray(a0, f32).reshape(1, 1), (128, 1)),
        "alm": np.tile(np.asarray(am, f32).reshape(1, 1), (128, 1)),
    }
    for l, (g, bt, p) in enumerate(((g1, bt1, p1), (g2, bt2, p2),
                                    (g3, bt3, p3)), start=1):
        com[f"g{l}c"] = np.ascontiguousarray(g.reshape(NDT, 128).T, dtype=f32)
        com[f"bt{l}c"] = np.ascontiguousarray(bt.reshape(NDT, 128).T, dtype=f32)
        com[f"al{l}"] = np.tile(np.asarray(p, f32).reshape(1, 1), (128, 1))

    # im2col: xp[(c,a,b), (s,i,j)] = x[s, c, 7i+a, 7j+b]
    xv = np.asarray(x, f32).reshape(B, CIN, IMG // KK, KK, IMG // KK, KK)
    perm = np.asarray(perm)
    in_maps = []
    for cix in range(N_CORES):
        xs = xv[cix * BL:(cix + 1) * BL]                     # [16,3,16,7,16,7]
        xp = np.ascontiguousarray(
            xs.transpose(1, 3, 5, 0, 2, 4).reshape(KP, T))
        ph = np.zeros((BL, 2, 2, 128, 128), f32)
        for s in range(BL):
            pg = perm[cix * BL + s].astype(np.int64)         # [256] token src idx
            to = np.arange(HW_)
            ph[s, pg // 128, to // 128, pg % 128, to % 128] = 1.0
        m = dict(com)
        m["xp"] = xp
        m["ph"] = ph
        in_maps.append(m)
    return in_maps


def kernel(**inputs):
    # BN bias b1..b3 cancel exactly under batch-norm mean subtraction; unused.
    for k in ("b1", "b2", "b3"):
        inputs.pop(k, None)
    if "nc" not in _cached:
        _cached["nc"] = _build()
    nc = _cached["nc"]
    in_maps = _prep_inputs(**inputs)
    trace = _cached.get("trace", False)
    res = run_bass_kernel_spmd(nc, in_maps, core_ids=list(range(N_CORES)),
                               trace=trace)
    _cached["last_result"] = res
    out = np.stack([r["out"] for r in res.results])          # [8, 4096, 512]
    return np.ascontiguousarray(out.reshape(B, HW_, HID), dtype=np.float32)
